# revision 14
# baseline (speedup 1.0000x reference)
"""Trainium2 Bass kernel for nn_AppearanceComposability (raw bass, manual sems).

Computation (per batch b, channel c, depth d):
    out[b,c,u,v,d] = (sum_{i=u..u+25, j=v..v+25} key[b,c,i,j,d]) * query[b,c,16,16,d]
with B=8, C=64, H=W=32, D=64, K=7 (window L=26). One batch per NeuronCore.

v2 architecture (from trace analysis of the v1 baseline at ~35.3us):
  Host folds q into x (commutes with the window sums), quantizes to fp8 e4m3
  with 2-D error diffusion, and pre-arranges to [(c4,i)=128 partitions, t,
  k, r, d] where c = 4t+c4, j = 2k+r, split into THREE dram tensors by k:
  xa (k 0:8), xb (k 8:14), xf (k 14:16) so every chunk is a contiguous DMA
  and the per-quad pipeline drains in small steps.

  At 8 concurrently-streaming cores each DGE path tops out at ~305 GB/s, but
  SWDGE+HWDGE together reach ~370 GB/s, so the input is split: xa chunks on
  gpsimd (SWDGE), xb+xf on sync (HWDGE-SP), a4 + output on ACT (HWDGE-ACT).

  Per quad (4 tiles on PE column groups, tile_position=(0,32g)):
    PE: banded block-diag stationary a4 [(c4,i) -> (c4,u)] contracts i.
      Pair sums via 2-deep psum accumulation (r=0 batch then r=1) into
      ps_m (k 0:14, bank-split at k=8) and ps_f (k 14:16); the odd-window
      boundary term u[m] = P[2m+1]+P[2m+26] accumulates in sps (3 batches).
    ACT evacuates psum to bf16 pb in three steps (k 0:8 / 8:14 / 14:16) as
      the matmuls complete, and issues the deferred output DMAs at the end.
    DVE assembles the 7 j-window sums with a 3-stage shifted-view tree
      pipelined against the evacs, so only ~5 small ops trail the last
      input byte.
  A few PE warmup matmuls run right after the NEFF preamble to lift the HAM
  clock gate; chunk pacing keeps PE gaps under the ~3.4us MID window.
  Output bf16; host casts/un-permutes to f32.

Raw bass with manual semaphores; every instruction carries at most one sem
wait (walrus rejects multi-wait instructions).
"""

from contextlib import ExitStack

import numpy as np

try:
    import concourse.bass as bass
except ImportError:
    import sys

    sys.path.insert(0, "/opt/trn_rl_repo")
    import concourse.bass as bass

from concourse import mybir

f32 = mybir.dt.float32
bf16 = mybir.dt.bfloat16
fp8 = mybir.dt.float8e4

B, C, H, W, D = 8, 64, 32, 32, 64
K = 7
L = H - K + 1  # 26
NT = C // 4  # 16 four-channel tiles
NQ = 4  # quads of 4 tiles
P = 128
KA, KB = 8, 6  # k 0:8 -> xa, 8:14 -> xb, 14:16 -> xf

# --- tunables ---------------------------------------------------------------
DT = "fp8"  # "fp8" | "bf16"
WARMUP = 6  # PE HAM warmup matmuls (N=512) while chunk xa0 streams in
KEEPALIVE = 2  # PE keepalive matmuls between quads (HAM insurance)
# ----------------------------------------------------------------------------


def build(dt=None):
    cdt = {"fp8": fp8, "bf16": bf16}[DT if dt is None else dt]

    nc = bass.Bass()
    # x[(c4,i), t, k, r, d]: j = 2k + r; k split 0:8 (xa) | 8:16 (xbf)
    xa = nc.declare_dram_parameter("xa", [P, NT, KA, 2, D], cdt, isOutput=False)
    xbf = nc.declare_dram_parameter("xbf", [P, NT, 8, 2, D], cdt, isOutput=False)
    a4 = nc.declare_dram_parameter("a4", [P, 4 * K], cdt, isOutput=False)
    # out blob: [P, Q, parity, m, d]; v = 2m + parity (parity=1, m=3 is pad)
    out = nc.declare_dram_parameter("out", [P, NQ, 2, 4, D], bf16, isOutput=True)

    ctx = ExitStack()
    with ctx:
        xa_sb = ctx.enter_context(nc.sbuf_tensor("xasb", [P, NT, KA, 2, D], cdt))
        xbf_sb = ctx.enter_context(nc.sbuf_tensor("xbfsb", [P, NT, 8, 2, D], cdt))
        a4_sb = ctx.enter_context(nc.sbuf_tensor("a4sb", [P, 4 * K], cdt))
        pbs = [
            ctx.enter_context(nc.sbuf_tensor(f"pb{i}", [P, 16, D], bf16))
            for i in range(3)
        ]
        ob = ctx.enter_context(nc.sbuf_tensor("ob", [P, NQ, 2, 4, D], bf16))
        e_s = ctx.enter_context(nc.sbuf_tensor("es", [P, 15, D], bf16))
        f_s = ctx.enter_context(nc.sbuf_tensor("fs", [P, 13, D], bf16))
        g_s = ctx.enter_context(nc.sbuf_tensor("gs", [P, 4, D], bf16))
        h_s = ctx.enter_context(nc.sbuf_tensor("hs", [P, 4, D], bf16))
        # psum: ps_m 2 banks x2 slots; F pairs + u share one bank x3 slots
        psm = [
            ctx.enter_context(nc.psum_tensor(f"psm{i}", [P, KA + KB, D], f32))
            for i in range(2)
        ]
        psfu = [
            ctx.enter_context(nc.psum_tensor(f"psfu{i}", [P, 5, D], f32))
            for i in range(3)
        ]
        warm_ps = ctx.enter_context(nc.psum_tensor("warmps", [P, 8, D], f32))

        lda4 = ctx.enter_context(nc.semaphore("lda4"))
        lda = ctx.enter_context(nc.semaphore("lda"))
        ldb = ctx.enter_context(nc.semaphore("ldb"))
        psem0 = ctx.enter_context(nc.semaphore("psem0"))
        psemA = ctx.enter_context(nc.semaphore("psemA"))
        psemB = ctx.enter_context(nc.semaphore("psemB"))
        ssem = ctx.enter_context(nc.semaphore("ssem"))
        wsem = ctx.enter_context(nc.semaphore("wsem"))
        vsem = ctx.enter_context(nc.semaphore("vsem"))
        osem = ctx.enter_context(nc.semaphore("osem"))

        last_wait = {}

        def wge(engine, ename, sem, val):
            key = (ename, id(sem))
            if last_wait.get(key, -1) < val:
                engine.wait_ge(sem, val)
                last_wait[key] = val

        with nc.Block(no_gpsimd_drain=True) as block:

            @block.sync
            def _(sync):
                for q in range(NQ):
                    sync.dma_start(
                        out=xa_sb[:, 4 * q : 4 * q + 4],
                        in_=xa[:, 4 * q : 4 * q + 4],
                    ).then_inc(lda, 16)
                # outputs: quads 0-2 batched, then quad 3 split by parity so
                # the even half streams while the stt still runs
                sync.wait_ge(vsem, 3)
                sync.dma_start(out=out[:, 0:3], in_=ob[:, 0:3]).then_inc(osem, 16)
                sync.wait_ge(wsem, 1)
                sync.dma_start(out=out[:, 3, 0:1], in_=ob[:, 3, 0:1]).then_inc(
                    osem, 16
                )
                sync.wait_ge(vsem, 4)
                sync.dma_start(out=out[:, 3, 1:2], in_=ob[:, 3, 1:2]).then_inc(
                    osem, 16
                )
                sync.wait_ge(osem, 48)

            @block.gpsimd
            def _(gp):
                for q in range(NQ):
                    gp.dma_start(
                        out=xbf_sb[:, 4 * q : 4 * q + 4],
                        in_=xbf[:, 4 * q : 4 * q + 4],
                    ).then_inc(ldb, 16)
                for q in range(NQ):
                    # the (parity=1, m=3) slot is pad the tree never writes;
                    # zero it so uninitialized SBUF can't leak NaNs out
                    nc.gpsimd.memset(ob[:, q, 1, 3, :], 0.0)

            @block.tensor
            def _(pe):
                def warm_mm():
                    # HAM warmup: garbage in, garbage out, own psum bank.
                    nc.tensor.matmul(
                        warm_ps[0:28],
                        xa_sb[:, 0, 0, 0, 0:28],
                        xa_sb[:, 0, 0:4, :, :],
                        start=True,
                        stop=True,
                        skip_group_check=True,
                    )

                for w in range(WARMUP):
                    warm_mm()
                wge(pe, "pe", lda4, 16)
                for q in range(NQ):
                    pm, pfu = psm[q % 2], psfu[q % 3]
                    if q >= 2:
                        # WAR: pair psum slot reuse after ACT evacs of q-2
                        wge(pe, "pe", ssem, 3 * q - 3)
                    if q >= 3:
                        # WAR: 3-slot psfu reuse after DVE stt of q-3
                        wge(pe, "pe", vsem, q - 2)
                    wge(pe, "pe", lda, 16 * (q + 1))

                    def rnd(dst, dk0, dk1, src_sb, sk0, sk1, r, start, stop,
                            sem=None):
                        for g in range(4):
                            mm = nc.tensor.matmul(
                                dst[32 * g : 32 * g + 28, dk0:dk1, :],
                                a4_sb[:],
                                src_sb[:, 4 * q + g, sk0:sk1, r, :],
                                start=start,
                                stop=stop,
                                tile_position=(0, 32 * g),
                                skip_group_check=True,
                            )
                        if sem is not None:
                            mm.then_inc(sem, 1)

                    # rounds of 4 col-group-parallel mms; accumulating pairs
                    # (r0 -> r1) are >=1 round apart so the psum RAW hides.
                    # u3 precedes F r0: start=True clears the whole bank's
                    # has_written bits, which would break a later accumulate.
                    rnd(pm, 0, KA, xa_sb, 0, KA, 0, True, False)  # A r0
                    rnd(pfu, 2, 5, xa_sb, 0, 3, 1, True, False)  # u1: j=1,3,5
                    rnd(pm, 0, KA, xa_sb, 0, KA, 1, False, True, psem0)  # A r1
                    wge(pe, "pe", ldb, 16 * (q + 1))
                    rnd(pm, KA, 14, xbf_sb, 0, KB, 0, True, False)  # B r0
                    rnd(pfu, 2, 3, xbf_sb, 5, 6, 0, False, True)  # u2: j=26
                    rnd(pm, KA, 14, xbf_sb, 0, KB, 1, False, True, psemA)
                    rnd(pfu, 3, 5, xbf_sb, 6, 8, 0, False, True)  # u3: j=28,30
                    rnd(pfu, 0, 2, xbf_sb, 6, 8, 0, True, False)  # F r0
                    rnd(pfu, 0, 2, xbf_sb, 6, 8, 1, False, True, psemB)  # F r1
                    if q < NQ - 1:
                        for w in range(KEEPALIVE):
                            warm_mm()

            @block.scalar
            def _(act):
                act.dma_start(out=a4_sb[:], in_=a4[:]).then_inc(lda4, 16)
                # touch the activation table now so the one-time
                # ACT_TABLE_LOAD (~1.3us) runs during the preamble window
                nc.scalar.copy(out=h_s[:, 0:1, 0:1], in_=e_s[:, 0:1, 0:1])
                for q in range(NQ):
                    pb = pbs[q % 3]
                    wge(act, "act", psem0, q + 1)
                    if q >= 3:
                        # WAR: 3-slot pb reused after DVE of quad q-3
                        wge(act, "act", vsem, q - 2)
                    nc.scalar.copy(
                        out=pb[:, 0:KA, :], in_=psm[q % 2][:, 0:KA, :]
                    ).then_inc(ssem, 1)
                    wge(act, "act", psemA, q + 1)
                    nc.scalar.copy(
                        out=pb[:, KA:14, :], in_=psm[q % 2][:, KA:14, :]
                    ).then_inc(ssem, 1)
                    wge(act, "act", psemB, q + 1)
                    nc.scalar.copy(
                        out=pb[:, 14:16, :], in_=psfu[q % 3][:, 0:2, :]
                    ).then_inc(ssem, 1)

            @block.vector
            def _(vec):
                for q in range(NQ):
                    pb = pbs[q % 3]
                    # stage 1: needs pairs k<8 only
                    wge(vec, "vec", ssem, 3 * q + 1)
                    nc.vector.tensor_add(
                        e_s[:, 0:7, :], pb[:, 0:7, :], pb[:, 1:8, :]
                    )
                    nc.vector.tensor_add(
                        f_s[:, 0:5, :], e_s[:, 0:5, :], e_s[:, 2:7, :]
                    )
                    # stage 2: needs pairs k<14
                    wge(vec, "vec", ssem, 3 * q + 2)
                    nc.vector.tensor_add(
                        e_s[:, 7:13, :], pb[:, 7:13, :], pb[:, 8:14, :]
                    )
                    nc.vector.drain()
                    nc.vector.tensor_add(
                        f_s[:, 5:11, :], e_s[:, 5:11, :], e_s[:, 7:13, :]
                    )
                    nc.vector.drain()
                    nc.vector.tensor_add(g_s[:], f_s[:, 0:4, :], f_s[:, 4:8, :])
                    nc.vector.drain()
                    nc.vector.tensor_add(
                        h_s[:, 0:3, :], g_s[:, 0:3, :], f_s[:, 8:11, :]
                    )
                    # stage 3: the k>=14 completion
                    wge(vec, "vec", ssem, 3 * q + 3)
                    nc.vector.tensor_add(
                        e_s[:, 13:15, :], pb[:, 13:15, :], pb[:, 14:16, :]
                    )
                    nc.vector.drain()
                    nc.vector.tensor_add(
                        f_s[:, 11:13, :], e_s[:, 11:13, :], e_s[:, 13:15, :]
                    )
                    nc.vector.drain()
                    nc.vector.tensor_add(
                        h_s[:, 3:4, :], g_s[:, 3:4, :], f_s[:, 11:12, :]
                    )
                    nc.vector.drain()
                    we = nc.vector.tensor_add(
                        ob[:, q, 0, :, :], h_s[:], pb[:, 12:16, :]
                    )
                    if q == NQ - 1:
                        we.then_inc(wsem, 1)
                    nc.vector.scalar_tensor_tensor(
                        ob[:, q, 1, 0:3, :],
                        h_s[:, 1:4, :],
                        0.0,
                        psfu[q % 3][:, 2:5, :],
                        mybir.AluOpType.add,
                        mybir.AluOpType.add,
                    ).then_inc(vsem, 1)

    return nc


def _host_inputs(key_map, query_map, dt=None):
    dtv = DT if dt is None else dt
    np_dt = mybir.dt.np(fp8 if dtv == "fp8" else bf16)

    a4 = np.zeros((P, 4 * K), dtype=np.float32)
    for c4 in range(4):
        for u in range(K):
            a4[c4 * 32 + u : c4 * 32 + u + L, c4 * K + u] = 1.0
    a4 = a4.astype(np_dt)

    key_map_f = np.asarray(key_map, dtype=np.float32)
    qc = np.asarray(query_map[:, :, H // 2, W // 2, :], dtype=np.float32)
    # q commutes with both window sums: fold it into x on the host.
    xq = key_map_f * qc[:, :, None, None, :]  # [B, C, H, W, D]

    if dtv == "fp8":
        # 2-D error diffusion (half right, half down): window-sum quantization
        # errors telescope to boundary terms.
        xl = np.ascontiguousarray(xq.transpose(0, 1, 4, 2, 3))  # [B,C,D,H,W]
        quant = np.empty_like(xl)
        carry_down = np.zeros(xl.shape[:3] + (W,), dtype=np.float32)
        for i in range(H):
            carry_right = np.zeros(xl.shape[:3], dtype=np.float32)
            nxt_down = np.empty_like(carry_down)
            for j in range(W):
                e = xl[..., i, j] + carry_right + carry_down[..., j]
                qe = e.astype(np_dt).astype(np.float32)
                r = e - qe
                carry_right = 0.5 * r
                nxt_down[..., j] = 0.5 * r
                quant[..., i, j] = qe
            carry_down = nxt_down
        xq = quant.transpose(0, 1, 3, 4, 2)  # back to [B,C,H,W,D]

    in_maps = []
    for b in range(B):
        xb_full = (
            xq[b]
            .reshape(NT, 4, H, W * D)
            .transpose(1, 2, 0, 3)  # [c4, i, t, (j d)]
            .reshape(P, NT, 16, 2, D)
            .astype(np_dt)
        )
        in_maps.append(
            {
                "xa": np.ascontiguousarray(xb_full[:, :, 0:KA]),
                "xbf": np.ascontiguousarray(xb_full[:, :, KA:16]),
                "a4": a4,
            }
        )
    return in_maps


def _host_output(blobs):
    # blob [P, Q, parity, m, d] -> out [B, C, K, K, D] f32
    full = np.empty((B, C, K, K, D), dtype=np.float32)
    for b in range(B):
        r = np.asarray(blobs[b], dtype=np.float32).reshape(4, 32, NQ, 2, 4, D)
        r = r[:, :28].reshape(4, 4, K, NQ, 2, 4, D)  # [g, c4, u, Q, par, m, d]
        for v in range(K):
            par, m = v % 2, v // 2
            # c = 16Q + 4g + c4
            full[b, :, :, v, :] = (
                r[:, :, :, :, par, m, :]
                .transpose(3, 0, 1, 2, 4)  # [Q, g, c4, u, d]
                .reshape(C, K, D)
            )
    return full


_cache = {}


def _get_nc():
    key = (DT, WARMUP, KEEPALIVE)
    if key not in _cache:
        _cache[key] = build()
    return _cache[key]


def kernel(key_map, query_map, _trace=False):
    from concourse.bass_utils import run_bass_kernel_spmd

    nc = _get_nc()
    in_maps = _host_inputs(key_map, query_map)
    res = run_bass_kernel_spmd(nc, in_maps, core_ids=list(range(B)), trace=_trace)
    out = _host_output([res.results[i]["out"] for i in range(B)])
    if _trace:
        return out, res
    return out


# revision 15
# speedup vs baseline: 1.0247x; 1.0247x over previous
"""Trainium2 Bass kernel for nn_AppearanceComposability (raw bass, manual sems).

Computation (per batch b, channel c, depth d):
    out[b,c,u,v,d] = (sum_{i=u..u+25, j=v..v+25} key[b,c,i,j,d]) * query[b,c,16,16,d]
with B=8, C=64, H=W=32, D=64, K=7 (window L=26). One batch per NeuronCore.

v2 architecture (from trace analysis of the v1 baseline at ~35.3us):
  Host folds q into x (commutes with the window sums), quantizes to fp8 e4m3
  with 2-D error diffusion, and pre-arranges to [(c4,i)=128 partitions, t,
  k, r, d] where c = 4t+c4, j = 2k+r, split into THREE dram tensors by k:
  xa (k 0:8), xb (k 8:14), xf (k 14:16) so every chunk is a contiguous DMA
  and the per-quad pipeline drains in small steps.

  At 8 concurrently-streaming cores each DGE path tops out at ~305 GB/s, but
  SWDGE+HWDGE together reach ~370 GB/s, so the input is split: xa chunks on
  gpsimd (SWDGE), xb+xf on sync (HWDGE-SP), a4 + output on ACT (HWDGE-ACT).

  Per quad (4 tiles on PE column groups, tile_position=(0,32g)):
    PE: banded block-diag stationary a4 [(c4,i) -> (c4,u)] contracts i.
      Pair sums via 2-deep psum accumulation (r=0 batch then r=1) into
      ps_m (k 0:14, bank-split at k=8) and ps_f (k 14:16); the odd-window
      boundary term u[m] = P[2m+1]+P[2m+26] accumulates in sps (3 batches).
    ACT evacuates psum to bf16 pb in three steps (k 0:8 / 8:14 / 14:16) as
      the matmuls complete, and issues the deferred output DMAs at the end.
    DVE assembles the 7 j-window sums with a 3-stage shifted-view tree
      pipelined against the evacs, so only ~5 small ops trail the last
      input byte.
  A few PE warmup matmuls run right after the NEFF preamble to lift the HAM
  clock gate; chunk pacing keeps PE gaps under the ~3.4us MID window.
  Output bf16; host casts/un-permutes to f32.

Raw bass with manual semaphores; every instruction carries at most one sem
wait (walrus rejects multi-wait instructions).
"""

from contextlib import ExitStack

import numpy as np

try:
    import concourse.bass as bass
except ImportError:
    import sys

    sys.path.insert(0, "/opt/trn_rl_repo")
    import concourse.bass as bass

from concourse import mybir

f32 = mybir.dt.float32
bf16 = mybir.dt.bfloat16
fp8 = mybir.dt.float8e4

B, C, H, W, D = 8, 64, 32, 32, 64
K = 7
L = H - K + 1  # 26
NT = C // 4  # 16 four-channel tiles
NQ = 4  # quads of 4 tiles
P = 128
KA, KB = 8, 6  # k 0:8 -> xa, 8:14 -> xb, 14:16 -> xf

# --- tunables ---------------------------------------------------------------
DT = "fp8"  # "fp8" | "bf16"
WARMUP = 6  # PE HAM warmup matmuls (N=512) while chunk xa0 streams in
KEEPALIVE = 2  # PE keepalive matmuls between quads (HAM insurance)
# ----------------------------------------------------------------------------


def build(dt=None):
    cdt = {"fp8": fp8, "bf16": bf16}[DT if dt is None else dt]

    nc = bass.Bass()
    # x[(c4,i), t, k, r, d]: j = 2k + r; k split 0:8 (xa) | 8:16 (xbf)
    xa = nc.declare_dram_parameter("xa", [P, NT, KA, 2, D], cdt, isOutput=False)
    xbf = nc.declare_dram_parameter("xbf", [P, NT, 8, 2, D], cdt, isOutput=False)
    a4 = nc.declare_dram_parameter("a4", [P, 4 * K], cdt, isOutput=False)
    # out blob: [P, Q, parity, m, d]; v = 2m + parity (parity=1, m=3 is pad)
    out = nc.declare_dram_parameter("out", [P, NQ, 2, 4, D], bf16, isOutput=True)

    ctx = ExitStack()
    with ctx:
        xa_sb = ctx.enter_context(nc.sbuf_tensor("xasb", [P, NT, KA, 2, D], cdt))
        xbf_sb = ctx.enter_context(nc.sbuf_tensor("xbfsb", [P, NT, 8, 2, D], cdt))
        a4_sb = ctx.enter_context(nc.sbuf_tensor("a4sb", [P, 4 * K], cdt))
        pbs = [
            ctx.enter_context(nc.sbuf_tensor(f"pb{i}", [P, 16, D], bf16))
            for i in range(3)
        ]
        ob = ctx.enter_context(nc.sbuf_tensor("ob", [P, NQ, 2, 4, D], bf16))
        e_s = ctx.enter_context(nc.sbuf_tensor("es", [P, 15, D], bf16))
        f_s = ctx.enter_context(nc.sbuf_tensor("fs", [P, 13, D], bf16))
        g_s = ctx.enter_context(nc.sbuf_tensor("gs", [P, 4, D], bf16))
        h_s = ctx.enter_context(nc.sbuf_tensor("hs", [P, 4, D], bf16))
        # psum: ps_m 2 banks x2 slots; F pairs + u share one bank x3 slots
        psm = [
            ctx.enter_context(nc.psum_tensor(f"psm{i}", [P, KA + KB, D], f32))
            for i in range(2)
        ]
        psfu = [
            ctx.enter_context(nc.psum_tensor(f"psfu{i}", [P, 5, D], f32))
            for i in range(3)
        ]
        warm_ps = ctx.enter_context(nc.psum_tensor("warmps", [P, 8, D], f32))

        lda4 = ctx.enter_context(nc.semaphore("lda4"))
        lda = ctx.enter_context(nc.semaphore("lda"))
        ldb = ctx.enter_context(nc.semaphore("ldb"))
        psem0 = ctx.enter_context(nc.semaphore("psem0"))
        psemA = ctx.enter_context(nc.semaphore("psemA"))
        psemB = ctx.enter_context(nc.semaphore("psemB"))
        ssem = ctx.enter_context(nc.semaphore("ssem"))
        wsem = ctx.enter_context(nc.semaphore("wsem"))
        vsem = ctx.enter_context(nc.semaphore("vsem"))
        osem = ctx.enter_context(nc.semaphore("osem"))

        last_wait = {}

        def wge(engine, ename, sem, val):
            key = (ename, id(sem))
            if last_wait.get(key, -1) < val:
                engine.wait_ge(sem, val)
                last_wait[key] = val

        with nc.Block(no_gpsimd_drain=True) as block:

            @block.sync
            def _(sync):
                sync.dma_start(out=a4_sb[:], in_=a4[:]).then_inc(lda4, 16)
                for q in range(NQ):
                    sync.dma_start(
                        out=xa_sb[:, 4 * q : 4 * q + 4],
                        in_=xa[:, 4 * q : 4 * q + 4],
                    ).then_inc(lda, 16)
                # outputs: quads 0-2 batched, then quad 3 split by parity so
                # the even half streams while the stt still runs
                sync.wait_ge(vsem, 3)
                sync.dma_start(out=out[:, 0:3], in_=ob[:, 0:3]).then_inc(osem, 16)
                sync.wait_ge(wsem, 1)
                sync.dma_start(out=out[:, 3, 0:1], in_=ob[:, 3, 0:1]).then_inc(
                    osem, 16
                )
                sync.wait_ge(vsem, 4)
                sync.dma_start(out=out[:, 3, 1:2], in_=ob[:, 3, 1:2]).then_inc(
                    osem, 16
                )
                sync.wait_ge(osem, 48)

            @block.gpsimd
            def _(gp):
                for q in range(NQ):
                    gp.dma_start(
                        out=xbf_sb[:, 4 * q : 4 * q + 4],
                        in_=xbf[:, 4 * q : 4 * q + 4],
                    ).then_inc(ldb, 16)
                for q in range(NQ):
                    # the (parity=1, m=3) slot is pad the tree never writes;
                    # zero it so uninitialized SBUF can't leak NaNs out
                    nc.gpsimd.memset(ob[:, q, 1, 3, :], 0.0)

            @block.tensor
            def _(pe):
                def warm_mm():
                    # HAM warmup: garbage in, garbage out, own psum bank.
                    nc.tensor.matmul(
                        warm_ps[0:28],
                        xa_sb[:, 0, 0, 0, 0:28],
                        xa_sb[:, 0, 0:4, :, :],
                        start=True,
                        stop=True,
                        skip_group_check=True,
                    )

                for w in range(WARMUP):
                    warm_mm()
                wge(pe, "pe", lda4, 16)
                for q in range(NQ):
                    pm, pfu = psm[q % 2], psfu[q % 3]
                    if q >= 2:
                        # WAR: pair psum slot reuse after ACT evacs of q-2
                        wge(pe, "pe", ssem, 3 * q - 3)
                    if q >= 3:
                        # WAR: 3-slot psfu reuse after DVE stt of q-3
                        wge(pe, "pe", vsem, q - 2)
                    wge(pe, "pe", lda, 16 * (q + 1))

                    def rnd(dst, dk0, dk1, src_sb, sk0, sk1, r, start, stop,
                            sem=None):
                        for g in range(4):
                            mm = nc.tensor.matmul(
                                dst[32 * g : 32 * g + 28, dk0:dk1, :],
                                a4_sb[:],
                                src_sb[:, 4 * q + g, sk0:sk1, r, :],
                                start=start,
                                stop=stop,
                                tile_position=(0, 32 * g),
                                skip_group_check=True,
                            )
                        if sem is not None:
                            mm.then_inc(sem, 1)

                    # rounds of 4 col-group-parallel mms; accumulating pairs
                    # (r0 -> r1) are >=1 round apart so the psum RAW hides.
                    # u3 precedes F r0: start=True clears the whole bank's
                    # has_written bits, which would break a later accumulate.
                    rnd(pm, 0, KA, xa_sb, 0, KA, 0, True, False)  # A r0
                    rnd(pfu, 2, 5, xa_sb, 0, 3, 1, True, False)  # u1: j=1,3,5
                    rnd(pm, 0, KA, xa_sb, 0, KA, 1, False, True, psem0)  # A r1
                    wge(pe, "pe", ldb, 16 * (q + 1))
                    rnd(pm, KA, 14, xbf_sb, 0, KB, 0, True, False)  # B r0
                    rnd(pfu, 2, 3, xbf_sb, 5, 6, 0, False, True)  # u2: j=26
                    rnd(pm, KA, 14, xbf_sb, 0, KB, 1, False, True, psemA)
                    rnd(pfu, 3, 5, xbf_sb, 6, 8, 0, False, True)  # u3: j=28,30
                    rnd(pfu, 0, 2, xbf_sb, 6, 8, 0, True, False)  # F r0
                    rnd(pfu, 0, 2, xbf_sb, 6, 8, 1, False, True, psemB)  # F r1
                    if q < NQ - 1:
                        for w in range(KEEPALIVE):
                            warm_mm()

            @block.scalar
            def _(act):
                # touch the activation table now so the one-time
                # ACT_TABLE_LOAD (~1.3us) runs during the preamble window
                nc.scalar.copy(out=h_s[:, 0:1, 0:1], in_=e_s[:, 0:1, 0:1])
                for q in range(NQ):
                    pb = pbs[q % 3]
                    wge(act, "act", psem0, q + 1)
                    if q >= 3:
                        # WAR: 3-slot pb reused after DVE of quad q-3
                        wge(act, "act", vsem, q - 2)
                    nc.scalar.copy(
                        out=pb[:, 0:KA, :], in_=psm[q % 2][:, 0:KA, :]
                    ).then_inc(ssem, 1)
                    wge(act, "act", psemA, q + 1)
                    nc.scalar.copy(
                        out=pb[:, KA:14, :], in_=psm[q % 2][:, KA:14, :]
                    ).then_inc(ssem, 1)
                    wge(act, "act", psemB, q + 1)
                    nc.scalar.copy(
                        out=pb[:, 14:16, :], in_=psfu[q % 3][:, 0:2, :]
                    ).then_inc(ssem, 1)

            @block.vector
            def _(vec):
                for q in range(NQ):
                    pb = pbs[q % 3]
                    wge(vec, "vec", ssem, 3 * q + 3)
                    nc.vector.tensor_add(
                        e_s[:, 0:15, :], pb[:, 0:15, :], pb[:, 1:16, :]
                    )
                    nc.vector.tensor_add(
                        f_s[:], e_s[:, 0:13, :], e_s[:, 2:15, :]
                    )
                    nc.vector.tensor_add(g_s[:], f_s[:, 0:4, :], f_s[:, 4:8, :])
                    nc.vector.drain()
                    nc.vector.tensor_add(h_s[:], g_s[:], f_s[:, 8:12, :])
                    nc.vector.drain()
                    we = nc.vector.tensor_add(
                        ob[:, q, 0, :, :], h_s[:], pb[:, 12:16, :]
                    )
                    if q == NQ - 1:
                        we.then_inc(wsem, 1)
                    nc.vector.scalar_tensor_tensor(
                        ob[:, q, 1, 0:3, :],
                        h_s[:, 1:4, :],
                        0.0,
                        psfu[q % 3][:, 2:5, :],
                        mybir.AluOpType.add,
                        mybir.AluOpType.add,
                    ).then_inc(vsem, 1)

    return nc


def _host_inputs(key_map, query_map, dt=None):
    dtv = DT if dt is None else dt
    np_dt = mybir.dt.np(fp8 if dtv == "fp8" else bf16)

    a4 = np.zeros((P, 4 * K), dtype=np.float32)
    for c4 in range(4):
        for u in range(K):
            a4[c4 * 32 + u : c4 * 32 + u + L, c4 * K + u] = 1.0
    a4 = a4.astype(np_dt)

    key_map_f = np.asarray(key_map, dtype=np.float32)
    qc = np.asarray(query_map[:, :, H // 2, W // 2, :], dtype=np.float32)
    # q commutes with both window sums: fold it into x on the host.
    xq = key_map_f * qc[:, :, None, None, :]  # [B, C, H, W, D]

    if dtv == "fp8":
        # 2-D error diffusion (half right, half down): window-sum quantization
        # errors telescope to boundary terms.
        xl = np.ascontiguousarray(xq.transpose(0, 1, 4, 2, 3))  # [B,C,D,H,W]
        quant = np.empty_like(xl)
        carry_down = np.zeros(xl.shape[:3] + (W,), dtype=np.float32)
        for i in range(H):
            carry_right = np.zeros(xl.shape[:3], dtype=np.float32)
            nxt_down = np.empty_like(carry_down)
            for j in range(W):
                e = xl[..., i, j] + carry_right + carry_down[..., j]
                qe = e.astype(np_dt).astype(np.float32)
                r = e - qe
                carry_right = 0.5 * r
                nxt_down[..., j] = 0.5 * r
                quant[..., i, j] = qe
            carry_down = nxt_down
        xq = quant.transpose(0, 1, 3, 4, 2)  # back to [B,C,H,W,D]

    in_maps = []
    for b in range(B):
        xb_full = (
            xq[b]
            .reshape(NT, 4, H, W * D)
            .transpose(1, 2, 0, 3)  # [c4, i, t, (j d)]
            .reshape(P, NT, 16, 2, D)
            .astype(np_dt)
        )
        in_maps.append(
            {
                "xa": np.ascontiguousarray(xb_full[:, :, 0:KA]),
                "xbf": np.ascontiguousarray(xb_full[:, :, KA:16]),
                "a4": a4,
            }
        )
    return in_maps


def _host_output(blobs):
    # blob [P, Q, parity, m, d] -> out [B, C, K, K, D] f32
    full = np.empty((B, C, K, K, D), dtype=np.float32)
    for b in range(B):
        r = np.asarray(blobs[b], dtype=np.float32).reshape(4, 32, NQ, 2, 4, D)
        r = r[:, :28].reshape(4, 4, K, NQ, 2, 4, D)  # [g, c4, u, Q, par, m, d]
        for v in range(K):
            par, m = v % 2, v // 2
            # c = 16Q + 4g + c4
            full[b, :, :, v, :] = (
                r[:, :, :, :, par, m, :]
                .transpose(3, 0, 1, 2, 4)  # [Q, g, c4, u, d]
                .reshape(C, K, D)
            )
    return full


_cache = {}


def _get_nc():
    key = (DT, WARMUP, KEEPALIVE)
    if key not in _cache:
        _cache[key] = build()
    return _cache[key]


def kernel(key_map, query_map, _trace=False):
    from concourse.bass_utils import run_bass_kernel_spmd

    nc = _get_nc()
    in_maps = _host_inputs(key_map, query_map)
    res = run_bass_kernel_spmd(nc, in_maps, core_ids=list(range(B)), trace=_trace)
    out = _host_output([res.results[i]["out"] for i in range(B)])
    if _trace:
        return out, res
    return out


# revision 16
# speedup vs baseline: 1.1652x; 1.1371x over previous
"""Trainium2 Bass kernel for nn_AppearanceComposability (raw bass, manual sems).

Computation (per batch b, channel c, depth d):
    out[b,c,u,v,d] = (sum_{i=u..u+25, j=v..v+25} key[b,c,i,j,d]) * query[b,c,16,16,d]
with B=8, C=64, H=W=32, D=64, K=7 (window L=26). One batch per NeuronCore.

v2 architecture (from trace analysis of the v1 baseline at ~35.3us):
  Host folds q into x (commutes with the window sums), quantizes to fp8 e4m3
  with 2-D error diffusion, and pre-arranges to [(c4,i)=128 partitions, t,
  k, r, d] where c = 4t+c4, j = 2k+r, split into THREE dram tensors by k:
  xa (k 0:8), xb (k 8:14), xf (k 14:16) so every chunk is a contiguous DMA
  and the per-quad pipeline drains in small steps.

  At 8 concurrently-streaming cores each DGE path tops out at ~305 GB/s, but
  SWDGE+HWDGE together reach ~370 GB/s, so the input is split: xa chunks on
  gpsimd (SWDGE), xb+xf on sync (HWDGE-SP), a4 + output on ACT (HWDGE-ACT).

  Per quad (4 tiles on PE column groups, tile_position=(0,32g)):
    PE: banded block-diag stationary a4 [(c4,i) -> (c4,u)] contracts i.
      Pair sums via 2-deep psum accumulation (r=0 batch then r=1) into
      ps_m (k 0:14, bank-split at k=8) and ps_f (k 14:16); the odd-window
      boundary term u[m] = P[2m+1]+P[2m+26] accumulates in sps (3 batches).
    ACT evacuates psum to bf16 pb in three steps (k 0:8 / 8:14 / 14:16) as
      the matmuls complete, and issues the deferred output DMAs at the end.
    DVE assembles the 7 j-window sums with a 3-stage shifted-view tree
      pipelined against the evacs, so only ~5 small ops trail the last
      input byte.
  A few PE warmup matmuls run right after the NEFF preamble to lift the HAM
  clock gate; chunk pacing keeps PE gaps under the ~3.4us MID window.
  Output bf16; host casts/un-permutes to f32.

Raw bass with manual semaphores; every instruction carries at most one sem
wait (walrus rejects multi-wait instructions).
"""

from contextlib import ExitStack

import numpy as np

try:
    import concourse.bass as bass
except ImportError:
    import sys

    sys.path.insert(0, "/opt/trn_rl_repo")
    import concourse.bass as bass

from concourse import mybir

f32 = mybir.dt.float32
bf16 = mybir.dt.bfloat16
fp8 = mybir.dt.float8e4

B, C, H, W, D = 8, 64, 32, 32, 64
K = 7
L = H - K + 1  # 26
NT = C // 4  # 16 four-channel tiles
NQ = 4  # quads of 4 tiles
P = 128
KA, KB = 8, 6  # k 0:8 -> xa, 8:14 -> xb, 14:16 -> xf

# --- tunables ---------------------------------------------------------------
DT = "fp8"  # "fp8" | "bf16"
WARMUP = 6  # PE HAM warmup matmuls (N=512) while chunk 0 streams in
# ----------------------------------------------------------------------------


def build(dt=None):
    cdt = {"fp8": fp8, "bf16": bf16}[DT if dt is None else dt]

    nc = bass.Bass()
    # x[(c4,i), t, k, r, d]: j = 2k + r
    x = nc.declare_dram_parameter("x", [P, NT, 16, 2, D], cdt, isOutput=False)
    a4 = nc.declare_dram_parameter("a4", [P, 4 * K], cdt, isOutput=False)
    # out blob: [P, Q, parity, m, d]; v = 2m + parity (parity=1, m=3 is pad)
    out = nc.declare_dram_parameter("out", [P, NQ, 2, 4, D], bf16, isOutput=True)

    ctx = ExitStack()
    with ctx:
        x_sb = ctx.enter_context(nc.sbuf_tensor("xsb", [P, NT, 16, 2, D], cdt))
        a4_sb = ctx.enter_context(nc.sbuf_tensor("a4sb", [P, 4 * K], cdt))
        pbs = [
            ctx.enter_context(nc.sbuf_tensor(f"pb{i}", [P, 16, D], bf16))
            for i in range(3)
        ]
        ob = ctx.enter_context(nc.sbuf_tensor("ob", [P, NQ, 2, 4, D], bf16))
        e_s = ctx.enter_context(nc.sbuf_tensor("es", [P, 15, D], bf16))
        f_s = ctx.enter_context(nc.sbuf_tensor("fs", [P, 13, D], bf16))
        g_s = ctx.enter_context(nc.sbuf_tensor("gs", [P, 4, D], bf16))
        h_s = ctx.enter_context(nc.sbuf_tensor("hs", [P, 4, D], bf16))
        # psum: ps_m 2 banks x2 slots; F pairs + u share one bank x3 slots
        psm = [
            ctx.enter_context(nc.psum_tensor(f"psm{i}", [P, KA + KB, D], f32))
            for i in range(2)
        ]
        psfu = [
            ctx.enter_context(nc.psum_tensor(f"psfu{i}", [P, 5, D], f32))
            for i in range(3)
        ]
        warm_ps = ctx.enter_context(nc.psum_tensor("warmps", [P, 8, D], f32))

        lda4 = ctx.enter_context(nc.semaphore("lda4"))
        ld = ctx.enter_context(nc.semaphore("ld"))
        psem0 = ctx.enter_context(nc.semaphore("psem0"))
        psemA = ctx.enter_context(nc.semaphore("psemA"))
        psemB = ctx.enter_context(nc.semaphore("psemB"))
        ssem = ctx.enter_context(nc.semaphore("ssem"))
        wsem = ctx.enter_context(nc.semaphore("wsem"))
        vsem = ctx.enter_context(nc.semaphore("vsem"))
        osem = ctx.enter_context(nc.semaphore("osem"))

        last_wait = {}

        def wge(engine, ename, sem, val):
            key = (ename, id(sem))
            if last_wait.get(key, -1) < val:
                engine.wait_ge(sem, val)
                last_wait[key] = val

        with nc.Block(no_gpsimd_drain=True) as block:

            @block.sync
            def _(sync):
                for q in range(NQ):
                    sync.dma_start(
                        out=x_sb[:, 4 * q : 4 * q + 4],
                        in_=x[:, 4 * q : 4 * q + 4],
                    ).then_inc(ld, 16)
                # outputs: quads 0-2 batched, then quad 3 split by parity so
                # the even half streams while the stt still runs
                sync.wait_ge(vsem, 3)
                sync.dma_start(out=out[:, 0:3], in_=ob[:, 0:3]).then_inc(osem, 16)
                sync.wait_ge(wsem, 1)
                sync.dma_start(out=out[:, 3, 0:1], in_=ob[:, 3, 0:1]).then_inc(
                    osem, 16
                )
                sync.wait_ge(vsem, 4)
                sync.dma_start(out=out[:, 3, 1:2], in_=ob[:, 3, 1:2]).then_inc(
                    osem, 16
                )
                sync.wait_ge(osem, 48)

            @block.gpsimd
            def _(gp):
                gp.dma_start(out=a4_sb[:], in_=a4[:]).then_inc(lda4, 16)
                for q in range(NQ):
                    # the (parity=1, m=3) slot is pad the tree never writes;
                    # zero it so uninitialized SBUF can't leak NaNs out
                    nc.gpsimd.memset(ob[:, q, 1, 3, :], 0.0)

            @block.tensor
            def _(pe):
                def warm_mm():
                    # HAM warmup: garbage in, garbage out, own psum bank.
                    nc.tensor.matmul(
                        warm_ps[0:28],
                        x_sb[:, 0, 0, 0, 0:28],
                        x_sb[:, 0, 0:4, :, :],
                        start=True,
                        stop=True,
                        skip_group_check=True,
                    )

                for w in range(WARMUP):
                    warm_mm()
                wge(pe, "pe", lda4, 16)
                for q in range(NQ):
                    pm, pfu = psm[q % 2], psfu[q % 3]
                    if q >= 2:
                        # WAR: pair psum slot reuse after ACT evacs of q-2
                        wge(pe, "pe", ssem, 3 * q - 3)
                    if q >= 3:
                        # WAR: 3-slot psfu reuse after DVE stt of q-3
                        wge(pe, "pe", vsem, q - 2)
                    wge(pe, "pe", ld, 16 * (q + 1))

                    def rnd(dst, dk0, dk1, sk0, sk1, r, start, stop, sem=None):
                        for g in range(4):
                            mm = nc.tensor.matmul(
                                dst[32 * g : 32 * g + 28, dk0:dk1, :],
                                a4_sb[:],
                                x_sb[:, 4 * q + g, sk0:sk1, r, :],
                                start=start,
                                stop=stop,
                                tile_position=(0, 32 * g),
                                skip_group_check=True,
                            )
                        if sem is not None:
                            mm.then_inc(sem, 1)

                    # rounds of 4 col-group-parallel mms; accumulating pairs
                    # (r0 -> r1) are >=1 round apart so the psum RAW hides.
                    # u3 precedes F r0: start=True clears the whole bank's
                    # has_written bits, breaking any later accumulate there.
                    rnd(pm, 0, KA, 0, KA, 0, True, False)  # A r0
                    rnd(pfu, 2, 5, 0, 3, 1, True, False)  # u1: j=1,3,5
                    rnd(pm, 0, KA, 0, KA, 1, False, True, psem0)  # A r1
                    rnd(pm, KA, 14, KA, 14, 0, True, False)  # B r0
                    rnd(pfu, 2, 3, 13, 14, 0, False, True)  # u2: j=26
                    rnd(pm, KA, 14, KA, 14, 1, False, True, psemA)  # B r1
                    rnd(pfu, 3, 5, 14, 16, 0, False, True)  # u3: j=28,30
                    rnd(pfu, 0, 2, 14, 16, 0, True, False)  # F r0
                    rnd(pfu, 0, 2, 14, 16, 1, False, True, psemB)  # F r1

            @block.scalar
            def _(act):
                # touch the activation table now so the one-time
                # ACT_TABLE_LOAD (~1.3us) runs during the preamble window
                nc.scalar.copy(out=h_s[:, 0:1, 0:1], in_=e_s[:, 0:1, 0:1])
                for q in range(NQ):
                    pb = pbs[q % 3]
                    wge(act, "act", psem0, q + 1)
                    if q >= 3:
                        # WAR: 3-slot pb reused after DVE of quad q-3
                        wge(act, "act", vsem, q - 2)
                    nc.scalar.copy(
                        out=pb[:, 0:KA, :], in_=psm[q % 2][:, 0:KA, :]
                    ).then_inc(ssem, 1)
                    wge(act, "act", psemA, q + 1)
                    nc.scalar.copy(
                        out=pb[:, KA:14, :], in_=psm[q % 2][:, KA:14, :]
                    ).then_inc(ssem, 1)
                    wge(act, "act", psemB, q + 1)
                    nc.scalar.copy(
                        out=pb[:, 14:16, :], in_=psfu[q % 3][:, 0:2, :]
                    ).then_inc(ssem, 1)

            @block.vector
            def _(vec):
                for q in range(NQ):
                    pb = pbs[q % 3]
                    wge(vec, "vec", ssem, 3 * q + 3)
                    nc.vector.tensor_add(
                        e_s[:, 0:15, :], pb[:, 0:15, :], pb[:, 1:16, :]
                    )
                    nc.vector.tensor_add(
                        f_s[:], e_s[:, 0:13, :], e_s[:, 2:15, :]
                    )
                    nc.vector.tensor_add(g_s[:], f_s[:, 0:4, :], f_s[:, 4:8, :])
                    nc.vector.drain()
                    nc.vector.tensor_add(h_s[:], g_s[:], f_s[:, 8:12, :])
                    nc.vector.drain()
                    we = nc.vector.tensor_add(
                        ob[:, q, 0, :, :], h_s[:], pb[:, 12:16, :]
                    )
                    if q == NQ - 1:
                        we.then_inc(wsem, 1)
                    nc.vector.scalar_tensor_tensor(
                        ob[:, q, 1, 0:3, :],
                        h_s[:, 1:4, :],
                        0.0,
                        psfu[q % 3][:, 2:5, :],
                        mybir.AluOpType.add,
                        mybir.AluOpType.add,
                    ).then_inc(vsem, 1)

    return nc


def _host_inputs(key_map, query_map, dt=None):
    dtv = DT if dt is None else dt
    np_dt = mybir.dt.np(fp8 if dtv == "fp8" else bf16)

    a4 = np.zeros((P, 4 * K), dtype=np.float32)
    for c4 in range(4):
        for u in range(K):
            a4[c4 * 32 + u : c4 * 32 + u + L, c4 * K + u] = 1.0
    a4 = a4.astype(np_dt)

    key_map_f = np.asarray(key_map, dtype=np.float32)
    qc = np.asarray(query_map[:, :, H // 2, W // 2, :], dtype=np.float32)
    # q commutes with both window sums: fold it into x on the host.
    xq = key_map_f * qc[:, :, None, None, :]  # [B, C, H, W, D]

    if dtv == "fp8":
        # 2-D error diffusion (half right, half down): window-sum quantization
        # errors telescope to boundary terms.
        xl = np.ascontiguousarray(xq.transpose(0, 1, 4, 2, 3))  # [B,C,D,H,W]
        quant = np.empty_like(xl)
        carry_down = np.zeros(xl.shape[:3] + (W,), dtype=np.float32)
        for i in range(H):
            carry_right = np.zeros(xl.shape[:3], dtype=np.float32)
            nxt_down = np.empty_like(carry_down)
            for j in range(W):
                e = xl[..., i, j] + carry_right + carry_down[..., j]
                qe = e.astype(np_dt).astype(np.float32)
                r = e - qe
                carry_right = 0.5 * r
                nxt_down[..., j] = 0.5 * r
                quant[..., i, j] = qe
            carry_down = nxt_down
        xq = quant.transpose(0, 1, 3, 4, 2)  # back to [B,C,H,W,D]

    in_maps = []
    for b in range(B):
        xb_full = (
            xq[b]
            .reshape(NT, 4, H, W * D)
            .transpose(1, 2, 0, 3)  # [c4, i, t, (j d)]
            .reshape(P, NT, 16, 2, D)
            .astype(np_dt)
        )
        in_maps.append({"x": xb_full, "a4": a4})
    return in_maps


def _host_output(blobs):
    # blob [P, Q, parity, m, d] -> out [B, C, K, K, D] f32
    full = np.empty((B, C, K, K, D), dtype=np.float32)
    for b in range(B):
        r = np.asarray(blobs[b], dtype=np.float32).reshape(4, 32, NQ, 2, 4, D)
        r = r[:, :28].reshape(4, 4, K, NQ, 2, 4, D)  # [g, c4, u, Q, par, m, d]
        for v in range(K):
            par, m = v % 2, v // 2
            # c = 16Q + 4g + c4
            full[b, :, :, v, :] = (
                r[:, :, :, :, par, m, :]
                .transpose(3, 0, 1, 2, 4)  # [Q, g, c4, u, d]
                .reshape(C, K, D)
            )
    return full


_cache = {}


def _get_nc():
    key = (DT, WARMUP)
    if key not in _cache:
        _cache[key] = build()
    return _cache[key]


def kernel(key_map, query_map, _trace=False):
    from concourse.bass_utils import run_bass_kernel_spmd

    nc = _get_nc()
    in_maps = _host_inputs(key_map, query_map)
    res = run_bass_kernel_spmd(nc, in_maps, core_ids=list(range(B)), trace=_trace)
    out = _host_output([res.results[i]["out"] for i in range(B)])
    if _trace:
        return out, res
    return out


# revision 17
# speedup vs baseline: 1.1740x; 1.0076x over previous
"""Trainium2 Bass kernel for nn_AppearanceComposability (raw bass, manual sems).

Computation (per batch b, channel c, depth d):
    out[b,c,u,v,d] = (sum_{i=u..u+25, j=v..v+25} key[b,c,i,j,d]) * query[b,c,16,16,d]
with B=8, C=64, H=W=32, D=64, K=7 (window L=26). One batch per NeuronCore.

v2 architecture (from trace analysis of the v1 baseline at ~35.3us):
  Host folds q into x (commutes with the window sums), quantizes to fp8 e4m3
  with 2-D error diffusion, and pre-arranges to [(c4,i)=128 partitions, t,
  k, r, d] where c = 4t+c4, j = 2k+r, split into THREE dram tensors by k:
  xa (k 0:8), xb (k 8:14), xf (k 14:16) so every chunk is a contiguous DMA
  and the per-quad pipeline drains in small steps.

  At 8 concurrently-streaming cores each DGE path tops out at ~305 GB/s, but
  SWDGE+HWDGE together reach ~370 GB/s, so the input is split: xa chunks on
  gpsimd (SWDGE), xb+xf on sync (HWDGE-SP), a4 + output on ACT (HWDGE-ACT).

  Per quad (4 tiles on PE column groups, tile_position=(0,32g)):
    PE: banded block-diag stationary a4 [(c4,i) -> (c4,u)] contracts i.
      Pair sums via 2-deep psum accumulation (r=0 batch then r=1) into
      ps_m (k 0:14, bank-split at k=8) and ps_f (k 14:16); the odd-window
      boundary term u[m] = P[2m+1]+P[2m+26] accumulates in sps (3 batches).
    ACT evacuates psum to bf16 pb in three steps (k 0:8 / 8:14 / 14:16) as
      the matmuls complete, and issues the deferred output DMAs at the end.
    DVE assembles the 7 j-window sums with a 3-stage shifted-view tree
      pipelined against the evacs, so only ~5 small ops trail the last
      input byte.
  A few PE warmup matmuls run right after the NEFF preamble to lift the HAM
  clock gate; chunk pacing keeps PE gaps under the ~3.4us MID window.
  Output bf16; host casts/un-permutes to f32.

Raw bass with manual semaphores; every instruction carries at most one sem
wait (walrus rejects multi-wait instructions).
"""

from contextlib import ExitStack

import numpy as np

try:
    import concourse.bass as bass
except ImportError:
    import sys

    sys.path.insert(0, "/opt/trn_rl_repo")
    import concourse.bass as bass

from concourse import mybir

f32 = mybir.dt.float32
bf16 = mybir.dt.bfloat16
fp8 = mybir.dt.float8e4

B, C, H, W, D = 8, 64, 32, 32, 64
K = 7
L = H - K + 1  # 26
NT = C // 4  # 16 four-channel tiles
NQ = 4  # quads of 4 tiles
P = 128
KA, KB = 8, 6  # k 0:8 -> xa, 8:14 -> xb, 14:16 -> xf

# --- tunables ---------------------------------------------------------------
DT = "fp8"  # "fp8" | "bf16"
WARMUP = 6  # PE HAM warmup matmuls (N=512) while chunk 0 streams in
# ----------------------------------------------------------------------------


def build(dt=None):
    cdt = {"fp8": fp8, "bf16": bf16}[DT if dt is None else dt]

    nc = bass.Bass()
    # x[(c4,i), t, k, r, d]: j = 2k + r
    x = nc.declare_dram_parameter("x", [P, NT, 16, 2, D], cdt, isOutput=False)
    a4 = nc.declare_dram_parameter("a4", [P, 4 * K], cdt, isOutput=False)
    # out blob: [P, Q, parity, m, d]; v = 2m + parity (parity=1, m=3 is pad)
    out = nc.declare_dram_parameter("out", [P, NQ, 2, 4, D], bf16, isOutput=True)

    ctx = ExitStack()
    with ctx:
        x_sb = ctx.enter_context(nc.sbuf_tensor("xsb", [P, NT, 16, 2, D], cdt))
        a4_sb = ctx.enter_context(nc.sbuf_tensor("a4sb", [P, 4 * K], cdt))
        pbs = [
            ctx.enter_context(nc.sbuf_tensor(f"pb{i}", [P, 16, D], bf16))
            for i in range(3)
        ]
        ob = ctx.enter_context(nc.sbuf_tensor("ob", [P, NQ, 2, 4, D], bf16))
        e_s = ctx.enter_context(nc.sbuf_tensor("es", [P, 15, D], bf16))
        f_s = ctx.enter_context(nc.sbuf_tensor("fs", [P, 13, D], bf16))
        g_s = ctx.enter_context(nc.sbuf_tensor("gs", [P, 4, D], bf16))
        h_s = ctx.enter_context(nc.sbuf_tensor("hs", [P, 4, D], bf16))
        # psum: ps_m 2 banks x2 slots; F pairs + u share one bank x3 slots
        psm = [
            ctx.enter_context(nc.psum_tensor(f"psm{i}", [P, 16, D], f32))
            for i in range(2)
        ]
        sps = [
            ctx.enter_context(nc.psum_tensor(f"sps{i}", [P, 3, D], f32))
            for i in range(3)
        ]
        warm_ps = ctx.enter_context(nc.psum_tensor("warmps", [P, 8, D], f32))

        lda4 = ctx.enter_context(nc.semaphore("lda4"))
        ld = ctx.enter_context(nc.semaphore("ld"))
        psem0 = ctx.enter_context(nc.semaphore("psem0"))
        psemB = ctx.enter_context(nc.semaphore("psemB"))
        ssem = ctx.enter_context(nc.semaphore("ssem"))
        wsem = ctx.enter_context(nc.semaphore("wsem"))
        vsem = ctx.enter_context(nc.semaphore("vsem"))
        osem = ctx.enter_context(nc.semaphore("osem"))

        last_wait = {}

        def wge(engine, ename, sem, val):
            key = (ename, id(sem))
            if last_wait.get(key, -1) < val:
                engine.wait_ge(sem, val)
                last_wait[key] = val

        with nc.Block(no_gpsimd_drain=True) as block:

            @block.sync
            def _(sync):
                for q in range(NQ):
                    sync.dma_start(
                        out=x_sb[:, 4 * q : 4 * q + 4],
                        in_=x[:, 4 * q : 4 * q + 4],
                    ).then_inc(ld, 16)
                # outputs: quads 0-2 batched, then quad 3 split by parity so
                # the even half streams while the stt still runs
                sync.wait_ge(vsem, 3)
                sync.dma_start(out=out[:, 0:3], in_=ob[:, 0:3]).then_inc(osem, 16)
                sync.wait_ge(wsem, 1)
                sync.dma_start(out=out[:, 3, 0:1], in_=ob[:, 3, 0:1]).then_inc(
                    osem, 16
                )
                sync.wait_ge(osem, 48)

            @block.gpsimd
            def _(gp):
                gp.dma_start(out=a4_sb[:], in_=a4[:]).then_inc(lda4, 16)
                for q in range(NQ):
                    # the (parity=1, m=3) slot is pad the tree never writes;
                    # zero it so uninitialized SBUF can't leak NaNs out
                    nc.gpsimd.memset(ob[:, q, 1, 3, :], 0.0)

            @block.tensor
            def _(pe):
                def warm_mm():
                    # HAM warmup: garbage in, garbage out, own psum bank.
                    nc.tensor.matmul(
                        warm_ps[0:28],
                        x_sb[:, 0, 0, 0, 0:28],
                        x_sb[:, 0, 0:4, :, :],
                        start=True,
                        stop=True,
                        skip_group_check=True,
                    )

                for w in range(WARMUP):
                    warm_mm()
                wge(pe, "pe", lda4, 16)
                for q in range(NQ):
                    pm, pfu = psm[q % 2], sps[q % 3]
                    if q >= 2:
                        # WAR: pair psum slot reuse after ACT evacs of q-2
                        wge(pe, "pe", ssem, 2 * q - 2)
                    if q >= 3:
                        # WAR: 3-slot psfu reuse after DVE stt of q-3
                        wge(pe, "pe", vsem, q - 2)
                    wge(pe, "pe", ld, 16 * (q + 1))

                    def rnd(dst, dk0, dk1, sk0, sk1, r, start, stop, sem=None):
                        for g in range(4):
                            mm = nc.tensor.matmul(
                                dst[32 * g : 32 * g + 28, dk0:dk1, :],
                                a4_sb[:],
                                x_sb[:, 4 * q + g, sk0:sk1, r, :],
                                start=start,
                                stop=stop,
                                tile_position=(0, 32 * g),
                                skip_group_check=True,
                            )
                        if sem is not None:
                            mm.then_inc(sem, 1)

                    # rounds of 4 col-group-parallel mms; accumulating pairs
                    # (r0 -> r1) are >=1 round apart so the psum RAW hides.
                    # u3 precedes F r0: start=True clears the whole bank's
                    # has_written bits, breaking any later accumulate there.
                    rnd(pm, 0, KA, 0, KA, 0, True, False)  # A r0
                    rnd(pfu, 0, 3, 0, 3, 1, True, False)  # u1: j=1,3,5
                    rnd(pm, 0, KA, 0, KA, 1, False, True, psem0)  # A r1
                    rnd(pm, KA, 14, KA, 14, 0, True, False)  # B r0
                    rnd(pfu, 0, 1, 13, 14, 0, False, True)  # u2: j=26
                    rnd(pm, KA, 14, KA, 14, 1, False, True)  # B r1
                    rnd(pfu, 1, 3, 14, 16, 0, False, True)  # u3: j=28,30
                    rnd(pm, 14, 16, 14, 16, 0, True, False)  # F r0
                    rnd(pm, 14, 16, 14, 16, 1, False, True, psemB)  # F r1

            @block.scalar
            def _(act):
                # touch the activation table now so the one-time
                # ACT_TABLE_LOAD (~1.3us) runs during the preamble window
                nc.scalar.copy(out=h_s[:, 0:1, 0:1], in_=e_s[:, 0:1, 0:1])
                for q in range(NQ):
                    pb = pbs[q % 3]
                    wge(act, "act", psem0, q + 1)
                    if q >= 3:
                        # WAR: 3-slot pb reused after DVE of quad q-3
                        wge(act, "act", vsem, q - 2)
                    nc.scalar.copy(
                        out=pb[:, 0:KA, :], in_=psm[q % 2][:, 0:KA, :]
                    ).then_inc(ssem, 1)
                    wge(act, "act", psemB, q + 1)
                    nc.scalar.copy(
                        out=pb[:, KA:16, :], in_=psm[q % 2][:, KA:16, :]
                    ).then_inc(ssem, 1)
                # quad 3 odd-parity output from ACT, parallel with sync's
                wge(act, "act", vsem, 4)
                nc.scalar.drain()
                act.dma_start(out=out[:, 3, 1:2], in_=ob[:, 3, 1:2]).then_inc(
                    osem, 16
                )

            @block.vector
            def _(vec):
                for q in range(NQ):
                    pb = pbs[q % 3]
                    wge(vec, "vec", ssem, 2 * q + 2)
                    nc.vector.tensor_add(
                        e_s[:, 0:15, :], pb[:, 0:15, :], pb[:, 1:16, :]
                    )
                    nc.vector.tensor_add(
                        f_s[:], e_s[:, 0:13, :], e_s[:, 2:15, :]
                    )
                    nc.vector.tensor_add(g_s[:], f_s[:, 0:4, :], f_s[:, 4:8, :])
                    nc.vector.drain()
                    nc.vector.tensor_add(h_s[:], g_s[:], f_s[:, 8:12, :])
                    nc.vector.drain()
                    we = nc.vector.tensor_add(
                        ob[:, q, 0, :, :], h_s[:], pb[:, 12:16, :]
                    )
                    if q == NQ - 1:
                        we.then_inc(wsem, 1)
                    nc.vector.scalar_tensor_tensor(
                        ob[:, q, 1, 0:3, :],
                        h_s[:, 1:4, :],
                        0.0,
                        sps[q % 3][:],
                        mybir.AluOpType.add,
                        mybir.AluOpType.add,
                    ).then_inc(vsem, 1)

    return nc


def _host_inputs(key_map, query_map, dt=None):
    dtv = DT if dt is None else dt
    np_dt = mybir.dt.np(fp8 if dtv == "fp8" else bf16)

    a4 = np.zeros((P, 4 * K), dtype=np.float32)
    for c4 in range(4):
        for u in range(K):
            a4[c4 * 32 + u : c4 * 32 + u + L, c4 * K + u] = 1.0
    a4 = a4.astype(np_dt)

    key_map_f = np.asarray(key_map, dtype=np.float32)
    qc = np.asarray(query_map[:, :, H // 2, W // 2, :], dtype=np.float32)
    # q commutes with both window sums: fold it into x on the host.
    xq = key_map_f * qc[:, :, None, None, :]  # [B, C, H, W, D]

    if dtv == "fp8":
        # 2-D error diffusion (half right, half down): window-sum quantization
        # errors telescope to boundary terms.
        xl = np.ascontiguousarray(xq.transpose(0, 1, 4, 2, 3))  # [B,C,D,H,W]
        quant = np.empty_like(xl)
        carry_down = np.zeros(xl.shape[:3] + (W,), dtype=np.float32)
        for i in range(H):
            carry_right = np.zeros(xl.shape[:3], dtype=np.float32)
            nxt_down = np.empty_like(carry_down)
            for j in range(W):
                e = xl[..., i, j] + carry_right + carry_down[..., j]
                qe = e.astype(np_dt).astype(np.float32)
                r = e - qe
                carry_right = 0.5 * r
                nxt_down[..., j] = 0.5 * r
                quant[..., i, j] = qe
            carry_down = nxt_down
        xq = quant.transpose(0, 1, 3, 4, 2)  # back to [B,C,H,W,D]

    in_maps = []
    for b in range(B):
        xb_full = (
            xq[b]
            .reshape(NT, 4, H, W * D)
            .transpose(1, 2, 0, 3)  # [c4, i, t, (j d)]
            .reshape(P, NT, 16, 2, D)
            .astype(np_dt)
        )
        in_maps.append({"x": xb_full, "a4": a4})
    return in_maps


def _host_output(blobs):
    # blob [P, Q, parity, m, d] -> out [B, C, K, K, D] f32
    full = np.empty((B, C, K, K, D), dtype=np.float32)
    for b in range(B):
        r = np.asarray(blobs[b], dtype=np.float32).reshape(4, 32, NQ, 2, 4, D)
        r = r[:, :28].reshape(4, 4, K, NQ, 2, 4, D)  # [g, c4, u, Q, par, m, d]
        for v in range(K):
            par, m = v % 2, v // 2
            # c = 16Q + 4g + c4
            full[b, :, :, v, :] = (
                r[:, :, :, :, par, m, :]
                .transpose(3, 0, 1, 2, 4)  # [Q, g, c4, u, d]
                .reshape(C, K, D)
            )
    return full


_cache = {}


def _get_nc():
    key = (DT, WARMUP)
    if key not in _cache:
        _cache[key] = build()
    return _cache[key]


def kernel(key_map, query_map, _trace=False):
    from concourse.bass_utils import run_bass_kernel_spmd

    nc = _get_nc()
    in_maps = _host_inputs(key_map, query_map)
    res = run_bass_kernel_spmd(nc, in_maps, core_ids=list(range(B)), trace=_trace)
    out = _host_output([res.results[i]["out"] for i in range(B)])
    if _trace:
        return out, res
    return out
